# revision 55
# baseline (speedup 1.0000x reference)
import sys

sys.path.insert(0, "/opt/trn_rl_repo")

import numpy as np
import ml_dtypes

import concourse.bass as bass
import concourse.bacc as bacc
import concourse.tile as tile
from concourse.bass_utils import run_bass_kernel_spmd
from concourse import mybir

B, L, D, H = 2, 2048, 1024, 16
DH = 64          # dim per head
HPC = 4          # heads per core
CPC = HPC * DH   # feature cols per core = 256
NCORES = 8

MM_DT = "bfloat16"
NP_MM = ml_dtypes.bfloat16 if MM_DT == "bfloat16" else np.float32

_CACHE = {}


def build_nc(mm_dt: str):
    nc = bacc.Bacc()
    mm_dt = mybir.dt(mm_dt)
    fp32 = mybir.dt.float32

    xq = nc.declare_dram_parameter("xq", (D, L), mm_dt, isOutput=False)   # q[b].T
    xk = nc.declare_dram_parameter("xk", (D, L), mm_dt, isOutput=False)   # k[b].T
    xv = nc.declare_dram_parameter("xv", (D, L), mm_dt, isOutput=False)   # v[b].T
    wq = nc.declare_dram_parameter("wq", (D, CPC), mm_dt, isOutput=False)
    wk = nc.declare_dram_parameter("wk", (D, CPC), mm_dt, isOutput=False)
    wv = nc.declare_dram_parameter("wv", (D, CPC), mm_dt, isOutput=False)
    wo = nc.declare_dram_parameter("wo", (CPC, D), mm_dt, isOutput=False)
    bq = nc.declare_dram_parameter("bq", (CPC, 1), fp32, isOutput=False)
    bk = nc.declare_dram_parameter("bk", (CPC, 1), fp32, isOutput=False)
    y = nc.declare_dram_parameter("y", (L, D), fp32, isOutput=True)       # partial out

    from contextlib import ExitStack

    with ExitStack() as es:
        tc = es.enter_context(tile.TileContext(nc))
        # NOTE: bufs are per named tag
        xt_pool = es.enter_context(tc.tile_pool(name="xt", bufs=1))     # 3 tags [128,8,2048]
        w_pool = es.enter_context(tc.tile_pool(name="w", bufs=1))       # 3 tags [128,8,256]
        wo_pool = es.enter_context(tc.tile_pool(name="wo", bufs=1))     # 2 tags [128,1024]
        bias_pool = es.enter_context(tc.tile_pool(name="bias", bufs=1))
        qt_pool = es.enter_context(tc.tile_pool(name="qt", bufs=1))     # 2 tags [128,2048]
        kt_pool = es.enter_context(tc.tile_pool(name="kt", bufs=1))
        vn_pool = es.enter_context(tc.tile_pool(name="vn", bufs=1))     # [128,16,4,65]
        pt_pool = es.enter_context(tc.tile_pool(name="pt", bufs=6))     # [128,512]
        zr_pool = es.enter_context(tc.tile_pool(name="zr", bufs=3))     # [1,512]
        zbs_pool = es.enter_context(tc.tile_pool(name="zbs", bufs=3))   # [64,512]
        ot_pool = es.enter_context(tc.tile_pool(name="ot", bufs=1))     # 2 tags [128,2048]
        y_pool = es.enter_context(tc.tile_pool(name="ysb", bufs=4))     # [128,512]
        psA = es.enter_context(tc.tile_pool(name="psA", bufs=2, space="PSUM"))
        psS = es.enter_context(tc.tile_pool(name="psS", bufs=2, space="PSUM"))
        psOT = es.enter_context(tc.tile_pool(name="psOT", bufs=2, space="PSUM"))
        if True:
            # ---- load inputs (DMA queue order == consumption order) ---------
            # dc-halved loads: first 4 accumulation matmuls only need half
            # of w+x, so the PE starts ~2-3us earlier (deps are per-slice)
            wk_sb = w_pool.tile([128, 8, CPC], mm_dt, name="wk")
            wk_r = wk.rearrange("(dc p) c -> p dc c", p=128)
            xk_sb = xt_pool.tile([128, 8, L], mm_dt, name="xk")
            xk_r = xk.rearrange("(dc p) c -> p dc c", p=128)
            nc.sync.dma_start(out=wk_sb[:, 0:4, :], in_=wk_r[:, 0:4, :])
            nc.sync.dma_start(out=xk_sb[:, 0:4, 0:512], in_=xk_r[:, 0:4, 0:512])
            nc.sync.dma_start(out=wk_sb[:, 4:8, :], in_=wk_r[:, 4:8, :])
            nc.sync.dma_start(out=xk_sb[:, 4:8, 0:512], in_=xk_r[:, 4:8, 0:512])
            bk_sb = bias_pool.tile([128, 2], fp32, name="bk")
            nc.sync.dma_start(out=bk_sb, in_=bk.rearrange("(cc p) o -> p cc o", p=128))

            wq_sb = w_pool.tile([128, 8, CPC], mm_dt, name="wq")
            wq_r = wq.rearrange("(dc p) c -> p dc c", p=128)
            xq_sb = xt_pool.tile([128, 8, L], mm_dt, name="xq")
            xq_r = xq.rearrange("(dc p) c -> p dc c", p=128)
            nc.sync.dma_start(out=wq_sb[:, 0:4, :], in_=wq_r[:, 0:4, :])
            nc.sync.dma_start(out=xq_sb[:, 0:4, 0:512], in_=xq_r[:, 0:4, 0:512])
            nc.sync.dma_start(out=wq_sb[:, 4:8, :], in_=wq_r[:, 4:8, :])
            nc.sync.dma_start(out=xq_sb[:, 4:8, 0:512], in_=xq_r[:, 4:8, 0:512])
            bq_sb = bias_pool.tile([128, 2], fp32, name="bq")
            nc.sync.dma_start(out=bq_sb, in_=bq.rearrange("(cc p) o -> p cc o", p=128))

            wv_sb = w_pool.tile([128, 8, CPC], mm_dt, name="wv")
            nc.sync.dma_start(out=wv_sb, in_=wv.rearrange("(dc p) c -> p dc c", p=128))
            xv_sb = xt_pool.tile([128, 8, L], mm_dt, name="xv")
            xv_r = xv.rearrange("(dc p) c -> p dc c", p=128)
            nc.sync.dma_start(out=xv_sb[:, :, 0:512], in_=xv_r[:, :, 0:512])

            for ch in range(1, 4):
                sl = slice(512 * ch, 512 * ch + 512)
                nc.sync.dma_start(out=xk_sb[:, :, sl], in_=xk_r[:, :, sl])
                nc.sync.dma_start(out=xq_sb[:, :, sl], in_=xq_r[:, :, sl])
                nc.sync.dma_start(out=xv_sb[:, :, sl], in_=xv_r[:, :, sl])

            wo_sb = []
            for cc in range(2):
                t = wo_pool.tile([128, D], mm_dt, name=f"wo{cc}")
                nc.sync.dma_start(out=t, in_=wo[cc * 128:(cc + 1) * 128, :])
                wo_sb.append(t)

            # ---- stage A helpers (emitted chunk-wise, interleaved with B) ---
            qt_sb = [qt_pool.tile([128, L], mm_dt, name=f"qt{i}") for i in range(2)]
            kt_sb = [kt_pool.tile([128, L], mm_dt, name=f"kt{i}") for i in range(2)]
            # V natural layout: [128(lt-part), 16 lt, 4 head, 65] (col 64 = ones)
            v_sb = vn_pool.tile([128, 16, 4, 65], mm_dt)
            nc.vector.memset(v_sb[:, :, :, 64:65], 1.0)

            def emit_QK(dst, x_sb, w_sb, b_sb, lg):
                for cc in range(2):
                    ps = psA.tile([128, 512], fp32)
                    for dc in range(8):
                        nc.tensor.matmul(
                            ps,
                            w_sb[:, dc, cc * 128:(cc + 1) * 128],
                            x_sb[:, dc, lg * 512:(lg + 1) * 512],
                            start=(dc == 0),
                            stop=(dc == 7),
                        )
                    nc.vector.tensor_scalar_add(
                        out=dst[cc][:, lg * 512:(lg + 1) * 512],
                        in0=ps,
                        scalar1=b_sb[:, cc:cc + 1],
                    )

            def emit_V(lt):
                ps = psA.tile([128, CPC], fp32)
                for dc in range(8):
                    nc.tensor.matmul(
                        ps,
                        xv_sb[:, dc, lt * 128:(lt + 1) * 128],
                        wv_sb[:, dc, :],
                        start=(dc == 0),
                        stop=(dc == 7),
                    )
                nc.vector.tensor_copy(
                    out=v_sb[:, lt, :, 0:64],
                    in_=ps.rearrange("p (h d) -> p h d", d=64),
                )

            # prologue: just enough of A to start B(g4=0)
            emit_QK(kt_sb, xk_sb, wk_sb, bk_sb, 0)
            emit_QK(qt_sb, xq_sb, wq_sb, bq_sb, 0)
            for lt in range(4):
                emit_V(lt)

            # ---- stage B + C interleaved ------------------------------------
            ot_sb = [ot_pool.tile([128, L], mm_dt, name=f"ot{i}") for i in range(2)]
            y_view = y.rearrange("(lt p) c -> p lt c", p=128)

            def emit_C(g4, dve_only=False):
                # dve_only for mid-kernel C groups: ACT is saturated with exps
                # there, DVE has slack; final C(3) alternates (ACT idle then)
                for li in range(4):
                    lt = g4 * 4 + li
                    for dg in range(2):
                        ps = psA.tile([128, 512], fp32)
                        for cc in range(2):
                            nc.tensor.matmul(
                                ps,
                                ot_sb[cc][:, lt * 128:(lt + 1) * 128],
                                wo_sb[cc][:, dg * 512:(dg + 1) * 512],
                                start=(cc == 0),
                                stop=(cc == 1),
                            )
                        yt = y_pool.tile([128, 512], fp32)
                        if dve_only or dg == 0:
                            nc.vector.tensor_copy(out=yt, in_=ps)
                        else:
                            nc.scalar.activation(
                                out=yt, in_=ps,
                                func=mybir.ActivationFunctionType.Copy,
                                bias=0.0,
                            )
                        nc.sync.dma_start(
                            out=y_view[:, lt, dg * 512:(dg + 1) * 512],
                            in_=yt,
                        )

            for g4 in range(4):
                for h in range(HPC):
                    cc = h // 2
                    ro = (h % 2) * 64
                    nkt = g4 * 4 + 4
                    ot_ps = psOT.tile([65, 512], fp32)
                    pts = {}

                    def emit_S_pair(k0):
                        # two kt tiles share a [128,1024] PSUM pair; non-diag
                        # pairs get a single wide exp (saves ACT overhead)
                        diag = (k0 // 4 == g4)
                        st = psS.tile([128, 1024], fp32, name="st2")
                        for j in range(2):
                            kt = k0 + j
                            off = 128 * (kt % 4) if diag else 0
                            base = j * 512
                            nc.tensor.matmul(
                                st[:, base + off:base + 512],
                                kt_sb[cc][ro:ro + 64, kt * 128:(kt + 1) * 128],
                                qt_sb[cc][ro:ro + 64,
                                          g4 * 512 + off:(g4 + 1) * 512],
                                start=True,
                                stop=True,
                            )
                        pt = pt_pool.tile([128, 1024], mm_dt, name="pt2")
                        if not diag:
                            nc.scalar.activation(
                                out=pt,
                                in_=st,
                                func=mybir.ActivationFunctionType.Exp,
                                scale=0.125,
                            )
                        else:
                            for j in range(2):
                                kt = k0 + j
                                off = 128 * (kt % 4)
                                base = j * 512
                                nc.scalar.activation(
                                    out=pt[:, base + off:base + 512],
                                    in_=st[:, base + off:base + 512],
                                    func=mybir.ActivationFunctionType.Exp,
                                    scale=0.125,
                                )
                                # keep iff f - p - off >= 0. Cols >= off+128
                                # all-keep (skip); cols < off all-fill (zeroes
                                # the stale region the partial exp skipped).
                                w = off + 128
                                nc.gpsimd.affine_select(
                                    out=pt[:, base:base + w],
                                    in_=pt[:, base:base + w],
                                    compare_op=mybir.AluOpType.is_ge,
                                    fill=0.0,
                                    base=-off,
                                    channel_multiplier=-1,
                                    pattern=[[1, w]],
                                )
                        pts[k0] = pt[:, 0:512]
                        pts[k0 + 1] = pt[:, 512:1024]

                    def emit_P(kt):
                        nc.tensor.matmul(
                            ot_ps,
                            v_sb[:, kt, h, :],
                            pts.pop(kt),
                            start=(kt == 0),
                            stop=(kt == nkt - 1),
                        )

                    npair = nkt // 2
                    for kp in range(npair):
                        emit_S_pair(2 * kp)
                        if kp >= 1:
                            emit_P(2 * kp - 2)
                            emit_P(2 * kp - 1)
                    emit_P(nkt - 2)
                    emit_P(nkt - 1)

                    # divide by Z (row 64) -- off the PE entirely.
                    # NB: reciprocal_approx_fast reading PSUM directly is
                    # silently wrong; bounce the row through SBUF first.
                    zrow = zr_pool.tile([1, 512], fp32, name="zrow")
                    nc.vector.tensor_copy(out=zrow, in_=ot_ps[64:65, :])
                    zr = zr_pool.tile([1, 512], fp32, name="zr")
                    nc.vector.reciprocal_approx_fast(out=zr, in_=zrow)
                    zb = zbs_pool.tile([64, 512], fp32)
                    nc.gpsimd.partition_broadcast(out_ap=zb, in_ap=zr)
                    nc.vector.tensor_mul(
                        out=ot_sb[cc][ro:ro + 64, g4 * 512:(g4 + 1) * 512],
                        in0=ot_ps[0:64, :],
                        in1=zb,
                    )
                    # fillers: projection chunks for g4+1 keep the PE busy
                    # while this group's exp/affine/divide chains drain
                    if g4 < 3:
                        nx = g4 + 1
                        if h == 0:
                            emit_QK(kt_sb, xk_sb, wk_sb, bk_sb, nx)
                        elif h == 1:
                            emit_QK(qt_sb, xq_sb, wq_sb, bq_sb, nx)
                        elif h == 2:
                            emit_V(4 * nx)
                            emit_V(4 * nx + 1)
                        else:
                            emit_V(4 * nx + 2)
                            emit_V(4 * nx + 3)
                    if h == 2 and g4 > 0:
                        emit_C(g4 - 1)

            # final C group on the (now free) psS pair tiles: 4 units in
            # flight instead of 2, one wide evict + one DMA per L-tile
            for li in range(4):
                lt = 12 + li
                st = psS.tile([128, 1024], fp32, name="st2")
                for dg in range(2):
                    for cc in range(2):
                        nc.tensor.matmul(
                            st[:, dg * 512:(dg + 1) * 512],
                            ot_sb[cc][:, lt * 128:(lt + 1) * 128],
                            wo_sb[cc][:, dg * 512:(dg + 1) * 512],
                            start=(cc == 0),
                            stop=(cc == 1),
                        )
                yt = y_pool.tile([128, 1024], fp32, name="yt2")
                if li % 2 == 0:
                    nc.vector.tensor_copy(out=yt, in_=st)
                else:
                    nc.scalar.activation(
                        out=yt, in_=st,
                        func=mybir.ActivationFunctionType.Copy,
                        bias=0.0,
                    )
                nc.sync.dma_start(out=y_view[:, lt, :], in_=yt)

    nc.compile()
    return nc


def _get_nc(mm_dt: str):
    if mm_dt not in _CACHE:
        _CACHE[mm_dt] = build_nc(mm_dt)
    return _CACHE[mm_dt]


def kernel(q, k, v, mask, Wq, bq, Wk, bk, Wv, bv, Wo, bo, _trace=False):
    nc = _get_nc(MM_DT)

    in_maps = []
    for c in range(NCORES):
        b = c // 4
        g = c % 4
        s = slice(g * CPC, (g + 1) * CPC)
        in_maps.append({
            "xq": np.ascontiguousarray(q[b].T).astype(NP_MM),
            "xk": np.ascontiguousarray(k[b].T).astype(NP_MM),
            "xv": np.ascontiguousarray(v[b].T).astype(NP_MM),
            "wq": np.ascontiguousarray(Wq[:, s]).astype(NP_MM),
            "wk": np.ascontiguousarray(Wk[:, s]).astype(NP_MM),
            "wv": np.ascontiguousarray(Wv[:, s]).astype(NP_MM),
            "wo": np.ascontiguousarray(Wo[s, :]).astype(NP_MM),
            "bq": np.ascontiguousarray(bq[s]).reshape(CPC, 1).astype(np.float32),
            "bk": np.ascontiguousarray(bk[s]).reshape(CPC, 1).astype(np.float32),
        })

    res = run_bass_kernel_spmd(nc, in_maps, list(range(NCORES)), trace=_trace)

    # host gather: out[b] = sum_g y_core(b,g) + (bo + bv @ Wo)
    const = (bo + bv.astype(np.float64) @ Wo.astype(np.float64)).astype(np.float64)
    out = np.zeros((B, L, D), np.float64)
    for c in range(NCORES):
        out[c // 4] += res.results[c]["y"].astype(np.float64)
    out += const[None, None, :]
    kernel.last_exec_time_ns = res.exec_time_ns
    return out.astype(np.float32)
